# revision 31
# baseline (speedup 1.0000x reference)
"""ConcatCritic pair-grid MLP on 8 Trainium2 NeuronCores.

out[a, b] = W3 @ relu(W2 @ relu(W1 @ relu(Wx @ x[a] + Wy @ y[b] + b0) + b1) + b2) + b3

Sharding: rows (a) of the 512x512 score grid are split across 8 cores
(64 rows each); y-side projection and the MLP weights are replicated.
The first linear layer is separable: hx = Wx @ x.T + b0 and hy = Wy @ y.T
are rank-B projections computed on the host (0.1% of total FLOPs); the
device kernel does the broadcast-add + relu over the pair grid and the
three dense layers (99.9% of FLOPs).

Device-side per core (64 a-rows):
  constants in SBUF: hy [256, 512], W1T/W2T [256, 256], w3 [256], hxb [256, 64]
  for each a:
    h0[h, b] = relu(hy[h, b] + hxb[h, a])        (DVE tensor_scalar, 2 chunks)
    ps1[g, b] = sum_h W1T[h, g] * h0[h, b]       (4 matmuls -> PSUM [128, 1024])
    h1 = relu(ps1 + b1)                          (ACT/DVE)
    ps2[g, b] = sum_h W2T[h, g] * h1[h, b]       (4 matmuls)
    h2 = relu(ps2 + b2)                          (ACT/DVE)
    ps3[0, b] = sum_h w3[h] * h2[h, b]           (2 matmuls, M=1)
    out[a, :] = copy(ps3) -> SBUF -> DMA to DRAM
b3 is added on the host after the gather.
"""

import os

import numpy as np

import concourse.bass as bass
import concourse.mybir as mybir
import concourse.tile as tile
from concourse import bacc, bass_utils

B = 512
DIM = 128
HID = 256
N_CORES = 8
A_PER_CORE = B // N_CORES  # 64

# Matmul/activation storage dtype: "f32", "f32r" (full-rate fp32 matmul
# mode), or "bf16".
MATMUL_DT = os.environ.get("KERNEL_DT", "f32r")

_COMPILED: dict = {}


def _build(dt_mode: str, zero_b1: bool, zero_b2: bool):
    """Build + schedule + bacc-compile the SPMD program (same on all cores)."""
    f32 = mybir.dt.float32
    if dt_mode in ("bf16", "fp16"):
        # 16-bit matmul operands: full-rate PE (1 cycle/row) + FWL weight
        # loads + 16-bit DVE modes. fp16 has a 10-bit mantissa (~8x finer
        # than bf16) and our activations are O(1), well inside fp16 range.
        sb_dt = mybir.dt.bfloat16 if dt_mode == "bf16" else mybir.dt.float16
        hy_dt = sb_dt
        ps_dt = f32
        ps_bufs = 3
        group = 2  # software-pipeline pairs of a-rows
    elif dt_mode == "f32r":
        # fp32r: full-rate fp32 matmul mode. The verifier requires every
        # matmul operand to be *written* as float32r (rounded), so the
        # activation tiles and weight tensors are declared float32r.
        sb_dt = mybir.dt.float32r
        hy_dt = f32  # hy only feeds DVE, not matmul
        ps_dt = f32
        ps_bufs = 3
        group = 1
    else:
        sb_dt = f32
        hy_dt = f32
        ps_dt = f32
        ps_bufs = 3
        group = 1

    nc = bacc.Bacc("TRN2", target_bir_lowering=False, num_devices=N_CORES)

    hy_d = nc.dram_tensor("hy", [HID, B], hy_dt, kind="ExternalInput").ap()
    w1t_d = nc.dram_tensor("w1t", [HID, HID], sb_dt, kind="ExternalInput").ap()
    w2t_d = nc.dram_tensor("w2t", [HID, HID], sb_dt, kind="ExternalInput").ap()
    w3_d = nc.dram_tensor("w3", [128, 2], sb_dt, kind="ExternalInput").ap()
    hxb_dt = f32  # tensor_scalar requires fp32 scalar operands
    hxb_d = nc.dram_tensor(
        "hxb", [128, 2 * A_PER_CORE], hxb_dt, kind="ExternalInput"
    ).ap()
    b1_d = nc.dram_tensor("b1c", [128, 2], f32, kind="ExternalInput").ap()
    b2_d = nc.dram_tensor("b2c", [128, 2], f32, kind="ExternalInput").ap()
    out_d = nc.dram_tensor("out", [A_PER_CORE, B], f32, kind="ExternalOutput").ap()

    Relu = mybir.ActivationFunctionType.Relu
    Add = mybir.AluOpType.add
    Max = mybir.AluOpType.max

    with tile.TileContext(nc) as tc:
        with (
            tc.tile_pool(name="consts", bufs=1) as cpool,
            tc.tile_pool(name="h0p", bufs=6) as h0pool,
            tc.tile_pool(name="h1p", bufs=4) as h1pool,
            tc.tile_pool(name="h2p", bufs=7) as h2pool,
            tc.tile_pool(name="rowp", bufs=6) as rowpool,
            tc.tile_pool(name="psum", bufs=ps_bufs, space="PSUM") as pspool,
            tc.tile_pool(name="psum3", bufs=2, space="PSUM") as ps3pool,
        ):
            # --- constants (spread across engine DMA queues; critical-path
            # tensors for the first iterations go first on each queue) ---
            hy0 = cpool.tile([128, B], hy_dt, tag="hy0")
            hy1 = cpool.tile([128, B], hy_dt, tag="hy1")
            w1t0 = cpool.tile([128, HID], sb_dt, tag="w1t0")
            w1t1 = cpool.tile([128, HID], sb_dt, tag="w1t1")
            w2t0 = cpool.tile([128, HID], sb_dt, tag="w2t0")
            w2t1 = cpool.tile([128, HID], sb_dt, tag="w2t1")
            w3t = cpool.tile([128, 2], sb_dt, tag="w3t")
            hxb = cpool.tile([128, 2 * A_PER_CORE], hxb_dt, tag="hxb")
            b1t = cpool.tile([128, 2], f32, tag="b1t")
            b2t = cpool.tile([128, 2], f32, tag="b2t")
            nc.sync.dma_start(hxb[:], hxb_d[:])
            nc.sync.dma_start(hy0[:], hy_d[0:128, :])
            nc.sync.dma_start(hy1[:], hy_d[128:256, :])
            nc.scalar.dma_start(w1t0[:], w1t_d[0:128, :])
            nc.scalar.dma_start(w1t1[:], w1t_d[128:256, :])
            nc.scalar.dma_start(w2t0[:], w2t_d[0:128, :])
            nc.scalar.dma_start(w2t1[:], w2t_d[128:256, :])
            nc.scalar.dma_start(w3t[:], w3_d[:])
            nc.scalar.dma_start(b1t[:], b1_d[:])
            nc.scalar.dma_start(b2t[:], b2_d[:])

            # PE warm-up: ~3.5us of tiny matmuls during the const-DMA wait so
            # the HAM clock gate is already at 8/8 when the real stream starts
            # (saves the ~8-matmul cold ramp at 1.2 GHz).
            wm = cpool.tile([128, 64], sb_dt, tag="wm")
            nc.vector.memset(wm[:], 0.0)
            wmps = ps3pool.tile([128, B], f32, tag="ps3", name="wmps")
            for _ in range(60):
                nc.tensor.matmul(
                    wmps[0:64, 0:64], wm[:, 0:64], wm[:], start=True, stop=True
                )

            w1 = [w1t0, w1t1]
            w2 = [w2t0, w2t1]
            hy = [hy0, hy1]

            def relu_stage(use_act, ps, out_tile, bias_t, zero_bias):
                """relu(ps + bias) -> out_tile on ACT or DVE."""
                if zero_bias:
                    if use_act:
                        nc.scalar.activation(out_tile[:], ps[:], Relu)
                    else:
                        nc.vector.tensor_scalar(out_tile[:], ps[:], 0.0, None, Max)
                else:
                    for c in range(2):
                        sl = slice(c * B, (c + 1) * B)
                        if use_act:
                            nc.scalar.activation(
                                out_tile[:, sl], ps[:, sl], Relu,
                                bias=bias_t[:, c : c + 1],
                            )
                        else:
                            nc.vector.tensor_scalar(
                                out_tile[:, sl], ps[:, sl],
                                bias_t[:, c : c + 1], 0.0, Add, Max,
                            )

            def mm_layer(ps, wt, rhs_of):
                """4 matmuls: ps[:, g*B:(g+1)*B] += wt[h]^T-block @ rhs(h)."""
                for g in range(2):
                    gs = slice(g * B, (g + 1) * B)
                    for h in range(2):
                        nc.tensor.matmul(
                            ps[:, gs],
                            wt[h][:, g * 128 : (g + 1) * 128],
                            rhs_of(h),
                            start=(h == 0),
                            stop=(h == 1),
                        )

            # Modulo-scheduled software pipeline: in tick t the PE runs
            # L1(t+1), L2(t), L3(t-1) whose inputs were all produced in
            # earlier ticks, so the matmul stream never waits on this tick's
            # relu stages. DVE computes h0 (layer 0) and h2; ACT computes h1
            # and the psum3 -> SBUF row copy.
            h0s, h1s, h2s, ps1s, ps2s, ps3s = {}, {}, {}, {}, {}, {}

            def emit_h0(a):
                h0s[a] = [
                    h0pool.tile([128, B], sb_dt, tag=f"h0_{c}", name=f"h0_{c}_{a}")
                    for c in range(2)
                ]
                for c in range(2):
                    nc.vector.tensor_scalar(
                        h0s[a][c][:], hy[c][:],
                        hxb[:, c * A_PER_CORE + a : c * A_PER_CORE + a + 1],
                        0.0, Add, Max,
                    )

            def emit_l1(a):
                ps1s[a] = pspool.tile([128, 2 * B], ps_dt, tag="ps", name=f"ps1_{a}")
                mm_layer(ps1s[a], w1, lambda h: h0s[a][h][:])

            def emit_h1(a):
                h1s[a] = h1pool.tile([128, 2 * B], sb_dt, tag="h1", name=f"h1_{a}")
                relu_stage(True, ps1s[a], h1s[a], b1t, zero_b1)
                del ps1s[a]

            def emit_l2(a):
                ps2s[a] = pspool.tile([128, 2 * B], ps_dt, tag="ps", name=f"ps2_{a}")
                mm_layer(ps2s[a], w2, lambda h: h1s[a][:, h * B : (h + 1) * B])

            def emit_h2(a):
                # split across both engines: the two PSUM banks of ps2 can be
                # read by ACT and DVE in parallel
                h2s[a] = h2pool.tile([128, 2 * B], sb_dt, tag="h2", name=f"h2_{a}")
                ps = ps2s[a]
                if zero_b2:
                    nc.scalar.activation(h2s[a][:, 0:B], ps[:, 0:B], Relu)
                    nc.vector.tensor_scalar(
                        h2s[a][:, B : 2 * B], ps[:, B : 2 * B], 0.0, None, Max
                    )
                else:
                    nc.scalar.activation(
                        h2s[a][:, 0:B], ps[:, 0:B], Relu, bias=b2t[:, 0:1]
                    )
                    nc.vector.tensor_scalar(
                        h2s[a][:, B : 2 * B], ps[:, B : 2 * B],
                        b2t[:, 1:2], 0.0, Add, Max,
                    )
                del ps2s[a]

            # Layer 3 scores for 4 consecutive a-rows accumulate into one PSUM
            # bank at partitions {0,32,64,96} via tile_position column strips.
            # The 8 matmuls are emitted back-to-back so different strips can
            # overlap on the PE array. Only the group's first matmul carries
            # start=True (bank-wide has_written clear); later strips rely on
            # the per-element overwrite-where-unset / accumulate-where-set
            # semantics, which is safe because no other matmul clears the
            # bank mid-group. One ACT copy + one DMA then flush 4 rows.
            def emit_l3_batch(a0):
                g = a0 // 4
                ps3s[g] = ps3pool.tile([128, B], f32, tag="ps3", name=f"ps3_{g}")
                for j in range(4):
                    a = a0 + j
                    for h in range(2):
                        nc.tensor.matmul(
                            ps3s[g][32 * j : 32 * j + 1, :],
                            w3t[:, h : h + 1],
                            h2s[a][:, h * B : (h + 1) * B],
                            start=(h == 0),
                            stop=(h == 1),
                            tile_position=(0, 32 * j),
                        )
                    del h2s[a]

            def emit_out(g):
                row = rowpool.tile([128, B], f32, tag="row", name=f"row_{g}")
                nc.scalar.copy(row[0:97, :], ps3s[g][0:97, :])
                del ps3s[g]
                nc.sync.dma_start(out_d[4 * g : 4 * g + 4, :], row[0:97:32, :])

            A = A_PER_CORE
            emit_h0(0)
            emit_h0(1)
            emit_l1(0)
            emit_h1(0)
            for t in range(A):
                if t + 2 < A:
                    emit_h0(t + 2)
                if t + 1 < A:
                    emit_l1(t + 1)
                    emit_h1(t + 1)
                emit_l2(t)
                emit_h2(t)
                if t >= 4 and t % 4 == 0:
                    emit_l3_batch(t - 4)
                    emit_out((t - 4) // 4)
            emit_l3_batch(A - 4)
            emit_out((A - 4) // 4)

    nc.compile()
    return nc


def _prep_host(x, y, W0, b0, W1, W2, W3, dt_mode: str):
    """Host-side prep: first-layer projections, transposes, per-core shards."""
    xf = np.asarray(x, np.float32)
    yf = np.asarray(y, np.float32)
    Wx = np.asarray(W0[:, :DIM], np.float32)
    Wy = np.asarray(W0[:, DIM:], np.float32)
    # [HID, B] layouts, hidden on partitions
    hxb = Wx @ xf.T + np.asarray(b0, np.float32)[:, None]  # [256, 512]
    hy = Wy @ yf.T  # [256, 512]
    w1t = np.ascontiguousarray(np.asarray(W1, np.float32).T)  # [h, g]
    w2t = np.ascontiguousarray(np.asarray(W2, np.float32).T)
    w3p = np.ascontiguousarray(
        np.asarray(W3, np.float32).reshape(2, 128).T
    )  # [128, 2]

    if dt_mode in ("bf16", "fp16"):
        if dt_mode == "bf16":
            import ml_dtypes

            ldt = ml_dtypes.bfloat16
        else:
            ldt = np.float16
        cast = lambda a: np.ascontiguousarray(a.astype(ldt))
        hy_s, w1t_s, w2t_s, w3p_s = cast(hy), cast(w1t), cast(w2t), cast(w3p)
        hxb = hxb.astype(np.float32)
    else:
        hy_s, w1t_s, w2t_s, w3p_s = (
            np.ascontiguousarray(hy),
            w1t,
            w2t,
            w3p,
        )
        hxb = hxb.astype(np.float32)
    return hxb, hy_s, w1t_s, w2t_s, w3p_s


def _get_compiled(dt_mode, zero_b1, zero_b2):
    key = (dt_mode, zero_b1, zero_b2)
    if key not in _COMPILED:
        _COMPILED[key] = _build(dt_mode, zero_b1, zero_b2)
    return _COMPILED[key]


def run(inputs: dict, trace: bool = False, dt_mode: str | None = None):
    """Run on 8 cores; returns (out [512,512] fp32, BassKernelResults)."""
    dt_mode = dt_mode or MATMUL_DT
    x, y = inputs["x"], inputs["y"]
    W0, b0 = inputs["W0"], inputs["b0"]
    W1, b1 = inputs["W1"], np.asarray(inputs["b1"], np.float32)
    W2, b2 = inputs["W2"], np.asarray(inputs["b2"], np.float32)
    W3, b3 = inputs["W3"], np.asarray(inputs["b3"], np.float32)

    zero_b1 = bool(np.all(b1 == 0.0))
    zero_b2 = bool(np.all(b2 == 0.0))
    nc = _get_compiled(dt_mode, zero_b1, zero_b2)

    hxb, hy_s, w1t_s, w2t_s, w3p_s = _prep_host(x, y, W0, b0, W1, W2, W3, dt_mode)
    b1c = np.ascontiguousarray(b1.reshape(2, 128).T)
    b2c = np.ascontiguousarray(b2.reshape(2, 128).T)

    in_maps = []
    for c in range(N_CORES):
        sl = slice(c * A_PER_CORE, (c + 1) * A_PER_CORE)
        # hxb packed [128, 2*A]: chunk ch of hidden at cols ch*A..(ch+1)*A
        hxb_c = np.ascontiguousarray(
            hxb[:, sl].reshape(2, 128, A_PER_CORE).transpose(1, 0, 2).reshape(128, -1)
        )
        in_maps.append(
            {
                "hy": hy_s,
                "w1t": w1t_s,
                "w2t": w2t_s,
                "w3": w3p_s,
                "hxb": hxb_c,
                "b1c": b1c,
                "b2c": b2c,
            }
        )

    res = bass_utils.run_bass_kernel_spmd(
        nc, in_maps, core_ids=list(range(N_CORES)), trace=trace
    )
    out = np.concatenate([res.results[c]["out"] for c in range(N_CORES)], axis=0)
    out = out + float(b3[0])
    return out.astype(np.float32), res


def kernel(**inputs) -> np.ndarray:
    out, _ = run(inputs, trace=False)
    return out


if __name__ == "__main__":
    # quick self-exercise with random data
    rng = np.random.default_rng(0)
    ins = {
        "x": rng.standard_normal((B, DIM), dtype=np.float32),
        "y": rng.standard_normal((B, DIM), dtype=np.float32),
        "W0": rng.standard_normal((HID, 2 * DIM), dtype=np.float32) / 16.0,
        "b0": np.zeros(HID, np.float32),
        "W1": rng.standard_normal((HID, HID), dtype=np.float32) / 16.0,
        "b1": np.zeros(HID, np.float32),
        "W2": rng.standard_normal((HID, HID), dtype=np.float32) / 16.0,
        "b2": np.zeros(HID, np.float32),
        "W3": rng.standard_normal((1, HID), dtype=np.float32) / 16.0,
        "b3": np.zeros(1, np.float32),
    }
    out = kernel(**ins)
    print("out", out.shape, out.dtype, out[:2, :4])


# revision 32
# speedup vs baseline: 1.0623x; 1.0623x over previous
"""ConcatCritic pair-grid MLP on 8 Trainium2 NeuronCores.

out[a, b] = W3 @ relu(W2 @ relu(W1 @ relu(Wx @ x[a] + Wy @ y[b] + b0) + b1) + b2) + b3

Sharding: rows (a) of the 512x512 score grid are split across 8 cores
(64 rows each); y-side projection and the MLP weights are replicated.
The first linear layer is separable: hx = Wx @ x.T + b0 and hy = Wy @ y.T
are rank-B projections computed on the host (0.1% of total FLOPs); the
device kernel does the broadcast-add + relu over the pair grid and the
three dense layers (99.9% of FLOPs).

Device-side per core (64 a-rows):
  constants in SBUF: hy [256, 512], W1T/W2T [256, 256], w3 [256], hxb [256, 64]
  for each a:
    h0[h, b] = relu(hy[h, b] + hxb[h, a])        (DVE tensor_scalar, 2 chunks)
    ps1[g, b] = sum_h W1T[h, g] * h0[h, b]       (4 matmuls -> PSUM [128, 1024])
    h1 = relu(ps1 + b1)                          (ACT/DVE)
    ps2[g, b] = sum_h W2T[h, g] * h1[h, b]       (4 matmuls)
    h2 = relu(ps2 + b2)                          (ACT/DVE)
    ps3[0, b] = sum_h w3[h] * h2[h, b]           (2 matmuls, M=1)
    out[a, :] = copy(ps3) -> SBUF -> DMA to DRAM
b3 is added on the host after the gather.
"""

import os

import numpy as np

import concourse.bass as bass
import concourse.mybir as mybir
import concourse.tile as tile
from concourse import bacc, bass_utils

B = 512
DIM = 128
HID = 256
N_CORES = 8
A_PER_CORE = B // N_CORES  # 64

# Matmul/activation storage dtype: "f32", "f32r" (full-rate fp32 matmul
# mode), or "bf16".
MATMUL_DT = os.environ.get("KERNEL_DT", "f32r")

_COMPILED: dict = {}


def _build(dt_mode: str, zero_b1: bool, zero_b2: bool):
    """Build + schedule + bacc-compile the SPMD program (same on all cores)."""
    f32 = mybir.dt.float32
    if dt_mode in ("bf16", "fp16"):
        # 16-bit matmul operands: full-rate PE (1 cycle/row) + FWL weight
        # loads + 16-bit DVE modes. fp16 has a 10-bit mantissa (~8x finer
        # than bf16) and our activations are O(1), well inside fp16 range.
        sb_dt = mybir.dt.bfloat16 if dt_mode == "bf16" else mybir.dt.float16
        hy_dt = sb_dt
        ps_dt = f32
        ps_bufs = 3
        group = 2  # software-pipeline pairs of a-rows
    elif dt_mode == "f32r":
        # fp32r: full-rate fp32 matmul mode. The verifier requires every
        # matmul operand to be *written* as float32r (rounded), so the
        # activation tiles and weight tensors are declared float32r.
        sb_dt = mybir.dt.float32r
        hy_dt = f32  # hy only feeds DVE, not matmul
        ps_dt = f32
        ps_bufs = 3
        group = 1
    else:
        sb_dt = f32
        hy_dt = f32
        ps_dt = f32
        ps_bufs = 3
        group = 1

    nc = bacc.Bacc("TRN2", target_bir_lowering=False, num_devices=N_CORES)

    hy_d = nc.dram_tensor("hy", [HID, B], hy_dt, kind="ExternalInput").ap()
    w1t_d = nc.dram_tensor("w1t", [HID, HID], sb_dt, kind="ExternalInput").ap()
    w2t_d = nc.dram_tensor("w2t", [HID, HID], sb_dt, kind="ExternalInput").ap()
    w3_d = nc.dram_tensor("w3", [128, 2], sb_dt, kind="ExternalInput").ap()
    hxb_dt = f32  # tensor_scalar requires fp32 scalar operands
    hxb_d = nc.dram_tensor(
        "hxb", [128, 2 * A_PER_CORE], hxb_dt, kind="ExternalInput"
    ).ap()
    b1_d = nc.dram_tensor("b1c", [128, 2], f32, kind="ExternalInput").ap()
    b2_d = nc.dram_tensor("b2c", [128, 2], f32, kind="ExternalInput").ap()
    out_d = nc.dram_tensor("out", [A_PER_CORE, B], f32, kind="ExternalOutput").ap()

    Relu = mybir.ActivationFunctionType.Relu
    Add = mybir.AluOpType.add
    Max = mybir.AluOpType.max

    with tile.TileContext(nc) as tc:
        with (
            tc.tile_pool(name="consts", bufs=1) as cpool,
            tc.tile_pool(name="h0p", bufs=6) as h0pool,
            tc.tile_pool(name="h1p", bufs=4) as h1pool,
            tc.tile_pool(name="h2p", bufs=7) as h2pool,
            tc.tile_pool(name="rowp", bufs=6) as rowpool,
            tc.tile_pool(name="psum", bufs=ps_bufs, space="PSUM") as pspool,
            tc.tile_pool(name="psum3", bufs=2, space="PSUM") as ps3pool,
        ):
            # --- constants (spread across engine DMA queues; critical-path
            # tensors for the first iterations go first on each queue) ---
            hy0 = cpool.tile([128, B], hy_dt, tag="hy0")
            hy1 = cpool.tile([128, B], hy_dt, tag="hy1")
            w1t0 = cpool.tile([128, HID], sb_dt, tag="w1t0")
            w1t1 = cpool.tile([128, HID], sb_dt, tag="w1t1")
            w2t0 = cpool.tile([128, HID], sb_dt, tag="w2t0")
            w2t1 = cpool.tile([128, HID], sb_dt, tag="w2t1")
            w3t = cpool.tile([128, 2], sb_dt, tag="w3t")
            hxb = cpool.tile([128, 2 * A_PER_CORE], hxb_dt, tag="hxb")
            b1t = cpool.tile([128, 2], f32, tag="b1t")
            b2t = cpool.tile([128, 2], f32, tag="b2t")
            nc.sync.dma_start(hxb[:], hxb_d[:])
            nc.sync.dma_start(hy0[:], hy_d[0:128, :])
            nc.sync.dma_start(hy1[:], hy_d[128:256, :])
            nc.scalar.dma_start(w1t0[:], w1t_d[0:128, :])
            nc.scalar.dma_start(w1t1[:], w1t_d[128:256, :])
            nc.scalar.dma_start(w2t0[:], w2t_d[0:128, :])
            nc.scalar.dma_start(w2t1[:], w2t_d[128:256, :])
            nc.scalar.dma_start(w3t[:], w3_d[:])
            nc.scalar.dma_start(b1t[:], b1_d[:])
            nc.scalar.dma_start(b2t[:], b2_d[:])

            # PE warm-up: ~3.5us of tiny matmuls during the const-DMA wait so
            # the HAM clock gate is already at 8/8 when the real stream starts
            # (saves the ~8-matmul cold ramp at 1.2 GHz).
            wm = cpool.tile([128, 64], sb_dt, tag="wm")
            nc.vector.memset(wm[:], 0.0)
            wmps = ps3pool.tile([128, B], f32, tag="ps3", name="wmps")
            for _ in range(60):
                nc.tensor.matmul(
                    wmps[0:64, 0:64], wm[:, 0:64], wm[:], start=True, stop=True
                )

            w1 = [w1t0, w1t1]
            w2 = [w2t0, w2t1]
            hy = [hy0, hy1]

            def relu_stage(use_act, ps, out_tile, bias_t, zero_bias):
                """relu(ps + bias) -> out_tile on ACT or DVE."""
                if zero_bias:
                    if use_act:
                        nc.scalar.activation(out_tile[:], ps[:], Relu)
                    else:
                        nc.vector.tensor_scalar(out_tile[:], ps[:], 0.0, None, Max)
                else:
                    for c in range(2):
                        sl = slice(c * B, (c + 1) * B)
                        if use_act:
                            nc.scalar.activation(
                                out_tile[:, sl], ps[:, sl], Relu,
                                bias=bias_t[:, c : c + 1],
                            )
                        else:
                            nc.vector.tensor_scalar(
                                out_tile[:, sl], ps[:, sl],
                                bias_t[:, c : c + 1], 0.0, Add, Max,
                            )

            def mm_layer(ps, wt, rhs_of):
                """4 matmuls: ps[:, g*B:(g+1)*B] += wt[h]^T-block @ rhs(h)."""
                for g in range(2):
                    gs = slice(g * B, (g + 1) * B)
                    for h in range(2):
                        nc.tensor.matmul(
                            ps[:, gs],
                            wt[h][:, g * 128 : (g + 1) * 128],
                            rhs_of(h),
                            start=(h == 0),
                            stop=(h == 1),
                        )

            # Modulo-scheduled software pipeline: in tick t the PE runs
            # L1(t+1), L2(t), L3(t-1) whose inputs were all produced in
            # earlier ticks, so the matmul stream never waits on this tick's
            # relu stages. DVE computes h0 (layer 0) and h2; ACT computes h1
            # and the psum3 -> SBUF row copy.
            h0s, h1s, h2s, ps1s, ps2s, ps3s = {}, {}, {}, {}, {}, {}

            def emit_h0(a):
                h0s[a] = [
                    h0pool.tile([128, B], sb_dt, tag=f"h0_{c}", name=f"h0_{c}_{a}")
                    for c in range(2)
                ]
                for c in range(2):
                    nc.vector.tensor_scalar(
                        h0s[a][c][:], hy[c][:],
                        hxb[:, c * A_PER_CORE + a : c * A_PER_CORE + a + 1],
                        0.0, Add, Max,
                    )

            def emit_l1(a):
                ps1s[a] = pspool.tile([128, 2 * B], ps_dt, tag="ps", name=f"ps1_{a}")
                mm_layer(ps1s[a], w1, lambda h: h0s[a][h][:])

            def emit_h1(a):
                h1s[a] = h1pool.tile([128, 2 * B], sb_dt, tag="h1", name=f"h1_{a}")
                relu_stage(True, ps1s[a], h1s[a], b1t, zero_b1)
                del ps1s[a]

            def emit_l2(a):
                ps2s[a] = pspool.tile([128, 2 * B], ps_dt, tag="ps", name=f"ps2_{a}")
                mm_layer(ps2s[a], w2, lambda h: h1s[a][:, h * B : (h + 1) * B])

            def emit_h2(a):
                h2s[a] = h2pool.tile([128, 2 * B], sb_dt, tag="h2", name=f"h2_{a}")
                relu_stage(False, ps2s[a], h2s[a], b2t, zero_b2)
                del ps2s[a]

            # Layer 3 scores for 4 consecutive a-rows accumulate into one PSUM
            # bank at partitions {0,32,64,96} via tile_position column strips.
            # The 8 matmuls are emitted back-to-back so different strips can
            # overlap on the PE array. Only the group's first matmul carries
            # start=True (bank-wide has_written clear); later strips rely on
            # the per-element overwrite-where-unset / accumulate-where-set
            # semantics, which is safe because no other matmul clears the
            # bank mid-group. One ACT copy + one DMA then flush 4 rows.
            def emit_l3_batch(a0):
                g = a0 // 4
                ps3s[g] = ps3pool.tile([128, B], f32, tag="ps3", name=f"ps3_{g}")
                for j in range(4):
                    a = a0 + j
                    for h in range(2):
                        nc.tensor.matmul(
                            ps3s[g][32 * j : 32 * j + 1, :],
                            w3t[:, h : h + 1],
                            h2s[a][:, h * B : (h + 1) * B],
                            start=(h == 0),
                            stop=(h == 1),
                            tile_position=(0, 32 * j),
                        )
                    del h2s[a]

            def emit_out(g):
                row = rowpool.tile([128, B], f32, tag="row", name=f"row_{g}")
                nc.scalar.copy(row[0:97, :], ps3s[g][0:97, :])
                del ps3s[g]
                nc.sync.dma_start(out_d[4 * g : 4 * g + 4, :], row[0:97:32, :])

            A = A_PER_CORE
            emit_h0(0)
            emit_h0(1)
            emit_l1(0)
            emit_h1(0)
            for t in range(A):
                if t + 2 < A:
                    emit_h0(t + 2)
                if t + 1 < A:
                    emit_l1(t + 1)
                    emit_h1(t + 1)
                emit_l2(t)
                emit_h2(t)
                if t >= 4 and t % 4 == 0:
                    emit_l3_batch(t - 4)
                    emit_out((t - 4) // 4)
            emit_l3_batch(A - 4)
            emit_out((A - 4) // 4)

    nc.compile()
    return nc


def _prep_host(x, y, W0, b0, W1, W2, W3, dt_mode: str):
    """Host-side prep: first-layer projections, transposes, per-core shards."""
    xf = np.asarray(x, np.float32)
    yf = np.asarray(y, np.float32)
    Wx = np.asarray(W0[:, :DIM], np.float32)
    Wy = np.asarray(W0[:, DIM:], np.float32)
    # [HID, B] layouts, hidden on partitions
    hxb = Wx @ xf.T + np.asarray(b0, np.float32)[:, None]  # [256, 512]
    hy = Wy @ yf.T  # [256, 512]
    w1t = np.ascontiguousarray(np.asarray(W1, np.float32).T)  # [h, g]
    w2t = np.ascontiguousarray(np.asarray(W2, np.float32).T)
    w3p = np.ascontiguousarray(
        np.asarray(W3, np.float32).reshape(2, 128).T
    )  # [128, 2]

    if dt_mode in ("bf16", "fp16"):
        if dt_mode == "bf16":
            import ml_dtypes

            ldt = ml_dtypes.bfloat16
        else:
            ldt = np.float16
        cast = lambda a: np.ascontiguousarray(a.astype(ldt))
        hy_s, w1t_s, w2t_s, w3p_s = cast(hy), cast(w1t), cast(w2t), cast(w3p)
        hxb = hxb.astype(np.float32)
    else:
        hy_s, w1t_s, w2t_s, w3p_s = (
            np.ascontiguousarray(hy),
            w1t,
            w2t,
            w3p,
        )
        hxb = hxb.astype(np.float32)
    return hxb, hy_s, w1t_s, w2t_s, w3p_s


def _get_compiled(dt_mode, zero_b1, zero_b2):
    key = (dt_mode, zero_b1, zero_b2)
    if key not in _COMPILED:
        _COMPILED[key] = _build(dt_mode, zero_b1, zero_b2)
    return _COMPILED[key]


def run(inputs: dict, trace: bool = False, dt_mode: str | None = None):
    """Run on 8 cores; returns (out [512,512] fp32, BassKernelResults)."""
    dt_mode = dt_mode or MATMUL_DT
    x, y = inputs["x"], inputs["y"]
    W0, b0 = inputs["W0"], inputs["b0"]
    W1, b1 = inputs["W1"], np.asarray(inputs["b1"], np.float32)
    W2, b2 = inputs["W2"], np.asarray(inputs["b2"], np.float32)
    W3, b3 = inputs["W3"], np.asarray(inputs["b3"], np.float32)

    zero_b1 = bool(np.all(b1 == 0.0))
    zero_b2 = bool(np.all(b2 == 0.0))
    nc = _get_compiled(dt_mode, zero_b1, zero_b2)

    hxb, hy_s, w1t_s, w2t_s, w3p_s = _prep_host(x, y, W0, b0, W1, W2, W3, dt_mode)
    b1c = np.ascontiguousarray(b1.reshape(2, 128).T)
    b2c = np.ascontiguousarray(b2.reshape(2, 128).T)

    in_maps = []
    for c in range(N_CORES):
        sl = slice(c * A_PER_CORE, (c + 1) * A_PER_CORE)
        # hxb packed [128, 2*A]: chunk ch of hidden at cols ch*A..(ch+1)*A
        hxb_c = np.ascontiguousarray(
            hxb[:, sl].reshape(2, 128, A_PER_CORE).transpose(1, 0, 2).reshape(128, -1)
        )
        in_maps.append(
            {
                "hy": hy_s,
                "w1t": w1t_s,
                "w2t": w2t_s,
                "w3": w3p_s,
                "hxb": hxb_c,
                "b1c": b1c,
                "b2c": b2c,
            }
        )

    res = bass_utils.run_bass_kernel_spmd(
        nc, in_maps, core_ids=list(range(N_CORES)), trace=trace
    )
    out = np.concatenate([res.results[c]["out"] for c in range(N_CORES)], axis=0)
    out = out + float(b3[0])
    return out.astype(np.float32), res


def kernel(**inputs) -> np.ndarray:
    out, _ = run(inputs, trace=False)
    return out


if __name__ == "__main__":
    # quick self-exercise with random data
    rng = np.random.default_rng(0)
    ins = {
        "x": rng.standard_normal((B, DIM), dtype=np.float32),
        "y": rng.standard_normal((B, DIM), dtype=np.float32),
        "W0": rng.standard_normal((HID, 2 * DIM), dtype=np.float32) / 16.0,
        "b0": np.zeros(HID, np.float32),
        "W1": rng.standard_normal((HID, HID), dtype=np.float32) / 16.0,
        "b1": np.zeros(HID, np.float32),
        "W2": rng.standard_normal((HID, HID), dtype=np.float32) / 16.0,
        "b2": np.zeros(HID, np.float32),
        "W3": rng.standard_normal((1, HID), dtype=np.float32) / 16.0,
        "b3": np.zeros(1, np.float32),
    }
    out = kernel(**ins)
    print("out", out.shape, out.dtype, out[:2, :4])
